# revision 45
# baseline (speedup 1.0000x reference)
"""Trainium2 Bass kernel for nn_CVRP_Encoder (AFT-style CVRP encoder).

Data-parallel over batch B=32 across 8 NeuronCores (4 items/core). Per item
everything lives in a transposed [D=128 (partitions), S=1000 (free)] layout so
instance-norm reduces along the free axis. S splits into 8 chunks of 125 for
the attention contraction (t on partitions) and 2 chunks of 500 for matmul
free dims.

Perf structure:
 - the big attention contractions (es@ek, es@ekv) run as fp8 DoubleRow
   matmuls (2 contraction rows per PE cycle): es in e4m3, ek/ekv in e5m2
   with a constant K-shift (ek' = exp(k - K0); e^-K0 cancels in num/den).
   The fp8 noise also largely cancels in num/den, measured ~0 extra error.
 - FFN stays f16 end-to-end (fp8 noise there hits the residual directly).
 - the residual stream is f16; norm chains use native scalar_tensor_tensor
   ops with fused sum accumulators: aft2 = (tanh(q/2)+1)*wgt (the sigmoid
   affine folded in), y = aft2*0.5 + x (accum sum(y)), y2' = A1*y + ff
   (accum sum(y2')).  C1 and bW2 are dropped: per-channel shifts cancel in
   the next instance norm (shift invariance).
 - sigmoid via tanh keeps every activation (tanh/exp/square/relu/identity)
   in the single `exp_and_others` hw table set: no table reloads.
 - relu passes split between scalar and vector engines; the normalized-x
   applications run on gpsimd (SBUF-only engine).
 - items are processed in norm-groups of 2, each group's norm chain emitted
   before the next group's elementwise work so it overlaps matmuls.
"""
import sys

sys.path.insert(0, "/opt/trn_rl_repo")

import numpy as np

import concourse.bass as bass
import concourse.tile as tile
from concourse import bacc, mybir
from concourse.bass_utils import run_bass_kernel_spmd

F32 = mybir.dt.float32
F16 = mybir.dt.float16
BF16 = mybir.dt.bfloat16
F8 = mybir.dt.float8e4
F8E5 = mybir.dt.float8e5
I32 = mybir.dt.int32
AF = mybir.ActivationFunctionType
ALU = mybir.AluOpType
DR = mybir.MatmulPerfMode.DoubleRow

B, N, D, F, L = 32, 999, 128, 512, 6
S = N + 1
P = 128
NCORES = 8
IPC = B // NCORES
TC = 8
TCS = S // TC      # 125
SC = 2
SCS = S // SC      # 500
FC = F // P        # 4
EPS = 1e-5
RSQRT_MAGIC = 0x5F3759DF + 1
GRP = 2            # items per norm-batching group
K0 = 3.5           # ek exponent shift (cancels in num/den)


def _bcast_dram(handle, n_part, idx, count):
    ap = handle[:]
    return bass.AP(tensor=ap.tensor, offset=idx, ap=[[0, n_part], [1, count]])


def _nv(t):
    """[P, 1024] tile/psum -> [P, 2, 500] strided view (skip 512-pad)."""
    return t[:].rearrange("p (n s) -> p n s", n=2)[:, :, 0:SCS]


def _v2(t):
    """[P, S] tile -> [P, 2, 500] view."""
    return t[:].rearrange("p (n s) -> p n s", n=2)


def _ckv(t):
    """[P, 1024] psum -> [P, 8, 128] chunk view."""
    return t[:].rearrange("p (c d) -> p c d", c=TC)


def build_cvrp(cs):
    """cs: per-layer scale constants c_l = log_scale * alpha[l]."""
    shared_es = all(abs(c - cs[0]) < 1e-30 for c in cs)

    nc = bacc.Bacc("TRN2", target_bir_lowering=False, debug=False,
                   num_devices=NCORES)

    g = {}
    g["dist8"] = nc.declare_dram_parameter("dist8", [IPC, TC, TCS, S], F8, isOutput=False)
    g["node_t"] = nc.declare_dram_parameter("node_t", [IPC, 3, N], F16, isOutput=False)
    g["depot"] = nc.declare_dram_parameter("depot", [IPC, 2], F32, isOutput=False)
    g["flagf"] = nc.declare_dram_parameter("flagf", [IPC], F32, isOutput=False)
    g["wqt"] = nc.declare_dram_parameter("wqt", [L, D, D], F16, isOutput=False)
    g["wkt"] = nc.declare_dram_parameter("wkt", [L, D, D], F16, isOutput=False)
    g["wvt"] = nc.declare_dram_parameter("wvt", [L, D, D], F16, isOutput=False)
    g["w1t"] = nc.declare_dram_parameter("w1t", [L, D, F], F16, isOutput=False)
    g["w2t"] = nc.declare_dram_parameter("w2t", [L, P, FC, D], F16, isOutput=False)
    g["wnt"] = nc.declare_dram_parameter("wnt", [3, D], F16, isOutput=False)
    g["wdt"] = nc.declare_dram_parameter("wdt", [2, D], F32, isOutput=False)
    g["wint"] = nc.declare_dram_parameter("wint", [D, D], F32, isOutput=False)
    g["woutt"] = nc.declare_dram_parameter("woutt", [D, D], F32, isOutput=False)
    g["biases4"] = nc.declare_dram_parameter("biases4", [D, 4], F32, isOutput=False)
    g["bw1_t"] = nc.declare_dram_parameter("bw1_t", [D, L, FC], F32, isOutput=False)
    g["g1_t"] = nc.declare_dram_parameter("g1_t", [D, L], F32, isOutput=False)
    g["b1_t"] = nc.declare_dram_parameter("b1_t", [D, L], F32, isOutput=False)
    g["g2_t"] = nc.declare_dram_parameter("g2_t", [D, L], F32, isOutput=False)
    g["b2_t"] = nc.declare_dram_parameter("b2_t", [D, L], F32, isOutput=False)
    g["out32"] = nc.declare_dram_parameter("out32", [IPC, D, S], F32, isOutput=True)

    with tile.TileContext(nc) as tc_ctx:
        _body(nc, tc_ctx, g, cs, shared_es)
    nc.compile()
    return nc


def _norm_smalls(nc, np_, sums, sumsq, g_col, b_col, tag, sum_scale, sq_scale,
                 sx=None, extras=None):
    """Instance-norm scalar math on [D, GRP] tiles.
    mean = (sums [+ sx]) * sum_scale; var = sumsq * sq_scale + eps - mean^2;
    rstd via bit-trick + 2 Newton iters. Returns (A, C): out = A*y + C for the
    TRUE-scale y. extras='n1' also returns Ah = A*0.5 (applies A to the 2x
    tensor); extras='n2' also returns (A2d, C2d) = (2A, 2C) and writes
    sx_next = A2d*sums + S*C2d into the provided column."""
    sm = np_.tile([D, 12, GRP], F32, tag=f"nsm_{tag}")
    mean, msq, var = sm[:, 0], sm[:, 1], sm[:, 2]
    if sx is not None:
        nc.vector.tensor_tensor(mean, sums, sx, ALU.add)
        nc.vector.tensor_scalar(mean, mean, sum_scale, None, ALU.mult)
    else:
        nc.vector.tensor_scalar(mean, sums, sum_scale, None, ALU.mult)
    nc.vector.tensor_tensor(msq, mean, mean, ALU.mult)
    nc.vector.tensor_scalar(var, sumsq, sq_scale, EPS, ALU.mult, ALU.add)
    nc.vector.tensor_tensor(var, var, msq, ALU.subtract)
    ry = sm[:, 3]
    ibits = ry.bitcast(I32)
    nc.vector.tensor_scalar(ibits, var.bitcast(I32), 1, -1,
                            ALU.logical_shift_right, ALU.bitwise_xor)
    nc.vector.tensor_scalar(ibits, ibits, RSQRT_MAGIC, None, ALU.add)
    t1, t2 = sm[:, 4], sm[:, 5]
    for _ in range(2):
        nc.vector.tensor_tensor(t1, ry, ry, ALU.mult)
        nc.vector.scalar_tensor_tensor(out=t2, in0=t1, scalar=-0.5, in1=var,
                                       op0=ALU.mult, op1=ALU.mult)
        nc.vector.scalar_tensor_tensor(out=ry, in0=t2, scalar=1.5, in1=ry,
                                       op0=ALU.add, op1=ALU.mult)
    A, C = sm[:, 6], sm[:, 7]
    nc.vector.tensor_scalar(A, ry, g_col, None, ALU.mult)
    nc.vector.tensor_tensor(C, mean, A, ALU.mult)
    nc.vector.tensor_scalar(C, C, b_col, -1.0, ALU.subtract, ALU.mult)
    if extras == "n1":
        Ah = sm[:, 8]
        nc.vector.tensor_scalar(Ah, A, 0.5, None, ALU.mult)
        return A, C, Ah
    if extras == "n2":
        A2d, C2d, sx_next, tmp = sm[:, 8], sm[:, 9], sm[:, 10], sm[:, 11]
        nc.vector.tensor_scalar(A2d, A, 2.0, None, ALU.mult)
        nc.vector.tensor_scalar(C2d, C, 2.0, None, ALU.mult)
        nc.vector.tensor_tensor(sx_next, A2d, sums, ALU.mult)
        nc.vector.tensor_scalar(tmp, C2d, float(S), None, ALU.mult)
        nc.vector.tensor_tensor(sx_next, sx_next, tmp, ALU.add)
        return A, C, A2d, C2d, sx_next
    return A, C


DBG = None      # e.g. ("y", 0): dump tile for each item at layer 0, then stop
DBG_STOP = [False]


def _dbg_dump(nc, g, xpool, name, l, i, ap):
    """Dump a 2D ap [p, f] (f<=1000) into out32[i][:p, :f] for debugging."""
    if DBG is None or DBG != (name, l):
        return
    p, f = ap.shape[0], ap.shape[-1]
    t = xpool.tile([D, S], F32, tag=f"dbg_{i}")
    nc.vector.tensor_scalar(t[:p, 0:f], ap, 1.0, None, ALU.mult)
    nc.sync.dma_start(g["out32"][i][:p, 0:f], t[:p, 0:f])
    if i == IPC - 1:
        DBG_STOP[0] = True


def _body(nc, tc, g, cs, shared_es):
    from contextlib import ExitStack

    ctx = ExitStack()
    singles = ctx.enter_context(tc.tile_pool(name="singles", bufs=1))
    xpool = ctx.enter_context(tc.tile_pool(name="xpool", bufs=1))
    tp = ctx.enter_context(tc.tile_pool(name="tp", bufs=2))
    scr = ctx.enter_context(tc.tile_pool(name="scr", bufs=2))
    np_ = ctx.enter_context(tc.tile_pool(name="npool", bufs=2))
    pp = ctx.enter_context(tc.tile_pool(name="pp", bufs=1))
    ps = ctx.enter_context(tc.tile_pool(name="ps", bufs=4, space="PSUM"))

    # ---- resident weights: L0-critical tensors on the SP queue (shortest
    # path), everything else on the Activation DMA queue so the startup
    # descriptor burst doesn't delay the embedding.
    t_wnt = singles.tile([3, D], F16, tag="wnt")
    nc.sync.dma_start(t_wnt[:], g["wnt"][:])
    t_wdt = singles.tile([2, D], F32, tag="wdt")
    nc.sync.dma_start(t_wdt[:], g["wdt"][:])
    t_wint = singles.tile([D, D], F32, tag="wint")
    nc.sync.dma_start(t_wint[:], g["wint"][:])
    t_woutt = singles.tile([D, D], F32, tag="woutt")
    nc.sync.dma_start(t_woutt[:], g["woutt"][:])
    sm_t = {}
    for nm, shp in (("biases4", [D, 4]), ("bw1_t", [D, L, FC]), ("g1_t", [D, L]),
                    ("b1_t", [D, L]), ("g2_t", [D, L]), ("b2_t", [D, L])):
        t = singles.tile(shp, F32, tag=nm)
        nc.sync.dma_start(t[:], g[nm][:])
        sm_t[nm] = t
    t_b4, t_bw1 = sm_t["biases4"], sm_t["bw1_t"]
    t_g1, t_b1, t_g2, t_b2 = sm_t["g1_t"], sm_t["b1_t"], sm_t["g2_t"], sm_t["b2_t"]
    t_ff = singles.tile([P, IPC], F32, tag="ffl")
    nc.sync.dma_start(t_ff[:], _bcast_dram(g["flagf"], P, 0, IPC))
    t_ksh = singles.tile([P, 1], F32, tag="kshift")
    nc.any.memset(t_ksh[:], -K0)
    t_wqt, t_wkt, t_wvt, t_w1t, t_w2t = [], [], [], [], []
    wspecs = ((t_wqt, g["wqt"], [D, D]), (t_wkt, g["wkt"], [D, D]),
              (t_wvt, g["wvt"], [D, D]), (t_w1t, g["w1t"], [D, F]),
              (t_w2t, g["w2t"], [P, FC, D]))
    for l in range(L):
        for lst, src, shape in wspecs:
            w = singles.tile(shape, F16, tag=f"w{id(lst)}_{l}")
            if l == 0:
                nc.sync.dma_start(w[:], src[l])
            lst.append(w)

    BD, BN_, BIN, BOUT = (t_b4[:, i : i + 1] for i in range(4))

    # ---- embedding ----
    x32s = []
    for i in range(IPC):
        x32 = xpool.tile([D, S], F32, tag=f"x32_{i}")
        t_node = scr.tile([P, 1024], F16, tag="node16")
        nc.sync.dma_start(t_node[:3, 0:N], g["node_t"][i])
        t_dep = tp.tile([2, 1], F32, tag="dep")
        nc.sync.dma_start(t_dep[:], g["depot"][i, :, None])
        pe = ps.tile([P, 1024], F32, tag="ps")
        nc.tensor.matmul(pe[:, 0:500], t_wnt[:], t_node[:3, 0:500], start=True, stop=True)
        nc.tensor.matmul(pe[:, 512:1011], t_wnt[:], t_node[:3, 500:999], start=True, stop=True)
        nc.scalar.activation(x32[:, 1:501], pe[:, 0:500], AF.Identity, bias=BN_, scale=1.0)
        nc.scalar.activation(x32[:, 501:1000], pe[:, 512:1011], AF.Identity, bias=BN_, scale=1.0)
        pd = ps.tile([P, 1024], F32, tag="ps")
        nc.tensor.matmul(pd[:, 0:1], t_wdt[:], t_dep[:], start=True, stop=True)
        nc.scalar.activation(x32[:, 0:1], pd[:, 0:1], AF.Identity, bias=BD, scale=1.0)
        pw = ps.tile([P, 1024], F32, tag="ps")
        nc.tensor.matmul(pw[:, 0:1], t_wint[:], x32[:, 1:2], start=True, stop=True)
        nc.scalar.activation(x32[:, 1:2], pw[:, 0:1], AF.Identity, bias=BIN, scale=1.0)
        # flag row fix: u = f*x0 + (1-f)*x999 ; w = Wout@u + bout ;
        # x0 += f*(w-u) ; x999 += (1-f)*(w-u)
        fcol = t_ff[:, i : i + 1]
        sm = np_.tile([D, 8], F32, tag="flagtmp")
        d1, u, t2, w_sb, d0 = (sm[:, j : j + 1] for j in range(5))
        nc.vector.tensor_tensor(d1, x32[:, 0:1], x32[:, 999:1000], ALU.subtract)
        nc.vector.tensor_scalar(d1, d1, fcol, None, ALU.mult)
        nc.vector.tensor_tensor(u, x32[:, 999:1000], d1, ALU.add)
        pf = ps.tile([P, 1024], F32, tag="ps")
        nc.tensor.matmul(pf[:, 0:1], t_woutt[:], u, start=True, stop=True)
        nc.scalar.activation(w_sb, pf[:, 0:1], AF.Identity, bias=BOUT, scale=1.0)
        nc.vector.tensor_tensor(t2, w_sb, u, ALU.subtract)
        nc.vector.tensor_scalar(d0, t2, fcol, None, ALU.mult)
        nc.vector.tensor_tensor(x32[:, 0:1], x32[:, 0:1], d0, ALU.add)
        nc.vector.tensor_tensor(x32[:, 999:1000], x32[:, 999:1000], t2, ALU.add)
        nc.vector.tensor_tensor(x32[:, 999:1000], x32[:, 999:1000], d0, ALU.subtract)
        x32s.append(x32)

    # The normalized stream is carried DOUBLED (x2x = 2x): the factor folds
    # into activation scales and norm constants for free, which lets the
    # y-residual add run as a plain gpsimd tensor_tensor.
    # sx_all[:, i] = sum_s(x2x) per item, carried for the norm-1 mean.
    sx_all = singles.tile([D, IPC], F32, tag="sxall")
    x16s = []
    for i in range(IPC):
        x16 = xpool.tile([D, S], F16, tag=f"x16_{i}")
        nc.gpsimd.tensor_scalar(x16[:], x32s[i][:], 2.0, None, ALU.mult)
        nc.vector.tensor_reduce(sx_all[:, i : i + 1], x32s[i][:],
                                axis=mybir.AxisListType.X, op=ALU.add)
        x16s.append(x16)
    nc.vector.tensor_scalar(sx_all[:], sx_all[:], 2.0, None, ALU.mult)

    for l in range(1, L):
        for li, (lst, src, shape) in enumerate(wspecs):
            nc.gpsimd.dma_start(lst[l][:], src[l])

    es_tiles = {}
    pending_norm2 = [None]

    # ---- encoder layers ----
    for l in range(L):
        sigs_tq, wgts, ys, h16s, pf2s, y2s, eks = {}, {}, {}, {}, {}, {}, {}
        sts = {}

        def attn_qkv(i):
            x16 = x16s[i]       # 2x-scaled stream; folded into scales below
            pq = ps.tile([P, 1024], F32, tag="ps")
            nc.tensor.matmul(pq[:, 0:500], t_wqt[l][:], x16[:, 0:500], start=True, stop=True)
            nc.tensor.matmul(pq[:, 512:1012], t_wqt[l][:], x16[:, 500:1000], start=True, stop=True)
            pk = ps.tile([P, 1024], F32, tag="ps")
            pv = ps.tile([P, 1024], F32, tag="ps")
            for c in range(TC):
                lhs = x16[:, c * TCS : (c + 1) * TCS]
                nc.tensor.matmul(pk[:TCS, c * P : (c + 1) * P], lhs, t_wkt[l][:], start=True, stop=True)
                nc.tensor.matmul(pv[:TCS, c * P : (c + 1) * P], lhs, t_wvt[l][:], start=True, stop=True)
            # K-shift: ek' = exp(k - K0); e^-K0 scales num and den identically.
            ek8 = tp.tile([P, TC, P], F8E5, tag=f"ek8_{i % 2}")
            nc.scalar.activation(ek8[:TCS], _ckv(pk)[:TCS], AF.Exp,
                                 bias=t_ksh[:TCS], scale=0.5)
            tq = pp.tile([P, S], F16, tag=f"tq_{i}")
            nc.scalar.activation(_v2(tq), _nv(pq), AF.Tanh, bias=0.0, scale=0.25)
            sigs_tq[i] = tq
            ekv8 = tp.tile([P, TC, P], F8E5, tag=f"ekv8_{i % 2}")
            nc.vector.scalar_tensor_tensor(
                out=ekv8[:TCS], in0=_ckv(pv)[:TCS], scalar=0.5, in1=ek8[:TCS],
                op0=ALU.mult, op1=ALU.mult)
            eks[i] = (ek8, ekv8)

        def attn_dennum(i):
            ek8, ekv8 = eks[i]
            if shared_es:
                if i not in es_tiles:
                    es = singles.tile([P, TC, S], F8, tag=f"es{i}", name=f"es{i}")
                    for h in range(SC):
                        hs = slice(h * SCS, (h + 1) * SCS)
                        nc.sync.dma_start(
                            es[:TCS, :, hs],
                            g["dist8"][i, :, :, hs].rearrange("c p s -> p c s"))
                        nc.scalar.activation(es[:TCS, :, hs], es[:TCS, :, hs],
                                             AF.Exp, bias=0.0, scale=-cs[0])
                    es_tiles[i] = es
                es = es_tiles[i]
            else:
                if i not in es_tiles:
                    raw = singles.tile([P, TC, S], F8, tag=f"esr{i}", name=f"esr{i}")
                    nc.sync.dma_start(raw[:TCS, 0:4], g["dist8"][i, 0:4].rearrange("c p s -> p c s"))
                    nc.sync.dma_start(raw[:TCS, 4:8], g["dist8"][i, 4:8].rearrange("c p s -> p c s"))
                    es_tiles[i] = raw
                es = tp.tile([P, TC, S], F8, tag="es_dyn")
                nc.scalar.activation(es[:TCS], es_tiles[i][:TCS], AF.Exp,
                                     bias=0.0, scale=-cs[l])
            pden = ps.tile([P, 1024], F32, tag="ps")
            for sc in range(SC):
                off = sc * 512
                ssl = slice(sc * SCS, (sc + 1) * SCS)
                for cp in range(TC // 2):
                    nc.tensor.matmul(pden[:, off : off + SCS],
                                     ek8[:TCS, 2 * cp : 2 * cp + 2, :],
                                     es[:TCS, 2 * cp : 2 * cp + 2, ssl],
                                     start=(cp == 0), stop=(cp == TC // 2 - 1),
                                     perf_mode=DR)
            pnum = ps.tile([P, 1024], F32, tag="ps")
            for sc in range(SC):
                off = sc * 512
                ssl = slice(sc * SCS, (sc + 1) * SCS)
                for cp in range(TC // 2):
                    nc.tensor.matmul(pnum[:, off : off + SCS],
                                     ekv8[:TCS, 2 * cp : 2 * cp + 2, :],
                                     es[:TCS, 2 * cp : 2 * cp + 2, ssl],
                                     start=(cp == 0), stop=(cp == TC // 2 - 1),
                                     perf_mode=DR)
            rden = scr.tile([P, 1024], F32, tag="rden")
            nc.vector.reciprocal_approx_fast(out=_nv(rden), in_=_nv(pden))
            wgt = pp.tile([P, S], F16, tag=f"wgt_{i}")
            nc.vector.tensor_tensor(_v2(wgt), _nv(pnum), _nv(rden), ALU.mult)
            wgts[i] = wgt
            _dbg_dump(nc, g, xpool, "x16", l, i, x16s[i][:])
            _dbg_dump(nc, g, xpool, "es", l, i, es[:TCS, 0, :])
            _dbg_dump(nc, g, xpool, "tq", l, i, sigs_tq[i][:])
            _dbg_dump(nc, g, xpool, "rden", l, i, rden[:, 0:500])
            _dbg_dump(nc, g, xpool, "wgt", l, i, wgt[:])

        def leafs(i):
            g0 = (i // GRP) * GRP
            if g0 not in sts:
                sts[g0] = np_.tile([D, 4, GRP], F32, tag=f"st_{g0}", name=f"st_{g0}")
            st1 = sts[g0]
            j = i - g0
            # aft2 = (tanh(q/2)+1)*wgt = 2*sigmoid(q)*weighted
            aft2 = tp.tile([P, S], F16, tag="aft")
            nc.vector.scalar_tensor_tensor(
                out=aft2[:], in0=sigs_tq[i][:], scalar=1.0, in1=wgts[i][:],
                op0=ALU.add, op1=ALU.mult, accum_out=st1[:, 0, j : j + 1])
            # y2x = 2y = aft2 + x2x  (sum comes analytically: Saft2 + sx)
            y = pp.tile([P, S], F16, tag=f"y_{i}")
            nc.gpsimd.tensor_tensor(y[:], aft2[:], x16s[i][:], ALU.add)
            ys[i] = y
            sq = scr.tile([P, 1024], F32, tag="scr4k")
            nc.scalar.activation(sq[:, 0:S], y[:], AF.Square, accum_out=st1[:, 1, j : j + 1])

        def norm1(g0):
            st1 = sts[g0]
            A1, C1, A1h = _norm_smalls(
                nc, np_, st1[:, 0], st1[:, 1],
                t_g1[:, l : l + 1], t_b1[:, l : l + 1], f"n1_{g0}",
                sum_scale=1.0 / (2 * S), sq_scale=1.0 / (4 * S),
                sx=sx_all[:, g0 : g0 + GRP], extras="n1")
            sts[f"AC{g0}"] = (A1, C1, A1h)
            for i in range(g0, g0 + GRP):
                j = i - g0
                h16 = pp.tile([P, S], F16, tag=f"h16_{i}")
                nc.gpsimd.tensor_scalar(h16[:], ys[i][:], A1h[:, j : j + 1], C1[:, j : j + 1],
                                        ALU.mult, ALU.add)
                h16s[i] = h16
                _dbg_dump(nc, g, xpool, "y", l, i, ys[i][:])
                _dbg_dump(nc, g, xpool, "h", l, i, h16[:])

        def ffn(i):
            h16 = h16s[i]
            pf2 = ps.tile([P, 1024], F32, tag="ps")
            r16 = tp.tile([P, FC, S], F16, tag="r16")
            r16v = r16[:].rearrange("p f (n s) -> p f n s", n=2)
            for fc in range(FC):
                pf1 = ps.tile([P, 1024], F32, tag="ps")
                w1 = t_w1t[l][:, fc * P : (fc + 1) * P]
                nc.tensor.matmul(pf1[:, 0:500], w1, h16[:, 0:500], start=True, stop=True)
                nc.tensor.matmul(pf1[:, 512:1012], w1, h16[:, 500:1000], start=True, stop=True)
                bcol = t_bw1[:, l, fc : fc + 1]
                if fc < 3:
                    nc.scalar.activation(r16v[:, fc], _nv(pf1), AF.Relu, bias=bcol, scale=1.0)
                else:
                    nc.vector.tensor_scalar(r16v[:, fc], _nv(pf1), bcol, 0.0, ALU.add, ALU.max)
            for sc in range(SC):
                off = sc * 512
                ssl = slice(sc * SCS, (sc + 1) * SCS)
                for fc in range(FC):
                    nc.tensor.matmul(pf2[:, off : off + SCS],
                                     t_w2t[l][:, fc, :], r16[:, fc, ssl],
                                     start=(fc == 0), stop=(fc == FC - 1))
            pf2s[i] = pf2

        def y2p(i):
            g0 = (i // GRP) * GRP
            st1 = sts[g0]
            A1, C1, A1h = sts[f"AC{g0}"]
            j = i - g0
            # y2' = A1*y + ff  (C1 and bW2 shifts cancel in this norm)
            y2 = pp.tile([P, S], F16, tag=f"y2_{i}")
            nc.vector.scalar_tensor_tensor(
                out=_v2(y2), in0=_v2(ys[i]), scalar=A1h[:, j : j + 1],
                in1=_nv(pf2s[i]), op0=ALU.mult, op1=ALU.add,
                accum_out=st1[:, 2, j : j + 1])
            y2s[i] = y2
            _dbg_dump(nc, g, xpool, "y2", l, i, y2[:])
            sq = scr.tile([P, 1024], F32, tag="scr4k")
            nc.scalar.activation(sq[:, 0:S], y2[:], AF.Square, accum_out=st1[:, 3, j : j + 1])

        def norm2(g0, sts=sts, y2s=y2s, l=l):
            st1 = sts[g0]
            A2, C2, A2d, C2d, sxn = _norm_smalls(
                nc, np_, st1[:, 2], st1[:, 3],
                t_g2[:, l : l + 1], t_b2[:, l : l + 1], f"n2_{g0}",
                sum_scale=1.0 / S, sq_scale=1.0 / S, extras="n2")
            nc.vector.tensor_scalar(sx_all[:, g0 : g0 + GRP], sxn, 1.0, None, ALU.mult)
            for i in range(g0, g0 + GRP):
                j = i - g0
                if l < L - 1:
                    nx16 = xpool.tile([D, S], F16, tag=f"x16_{i}{'b' if l % 2 == 0 else ''}")
                    nc.gpsimd.tensor_scalar(nx16[:], y2s[i][:], A2d[:, j : j + 1], C2d[:, j : j + 1],
                                            ALU.mult, ALU.add)
                    x16s[i] = nx16
                else:
                    xout = xpool.tile([D, S], F32, tag=f"x32_{i}")
                    nc.gpsimd.tensor_scalar(xout[:], y2s[i][:], A2[:, j : j + 1], C2[:, j : j + 1],
                                            ALU.mult, ALU.add)
                    nc.sync.dma_start(g["out32"][i], xout[:])

        if DBG_STOP[0]:
            break
        # Software-pipelined emission: all qkv matmuls run ahead of den/num so
        # the PE never starves; every cross-engine wait has independent work
        # queued in front of it on the same engine queue.
        lo_groups = [g0 for g0 in range(0, IPC // 2, GRP)]
        hi_groups = [g0 for g0 in range(IPC // 2, IPC, GRP)]
        attn_qkv(0); attn_qkv(1)
        if pending_norm2[0] is not None:
            pending_norm2[0]()          # prev layer norm2(hi) + x2x(2,3)
            pending_norm2[0] = None
        attn_dennum(0)
        attn_qkv(2)
        attn_dennum(1)
        attn_qkv(3)
        leafs(0)
        attn_dennum(2)
        leafs(1)
        for g0 in lo_groups:
            norm1(g0)
        attn_dennum(3)
        leafs(2); leafs(3)
        ffn(0)
        for g0 in hi_groups:
            norm1(g0)
        y2p(0)
        ffn(1)
        y2p(1)
        for g0 in lo_groups:
            norm2(g0)
        ffn(2); ffn(3)
        y2p(2); y2p(3)
        n2 = norm2
        pending_norm2[0] = (lambda n2=n2, gs=tuple(hi_groups): [n2(g0) for g0 in gs])

    if pending_norm2[0] is not None and not DBG_STOP[0]:
        pending_norm2[0]()
    ctx.close()


# ------------------------------------------------------------------
# host wrapper
# ------------------------------------------------------------------
_cache = {}


def _get_nc(cs_key):
    if cs_key not in _cache:
        _cache[cs_key] = build_cvrp(list(cs_key))
    return _cache[cs_key]


def prep_inputs(depot_xy, node_xy_demand, dist, log_scale, flag,
                Wd, bd, Wn, bn, Win, bin_, Wout, bout,
                Wq, Wk, Wv, alpha, g1, b1, W1, bW1, W2, bW2, g2, b2):
    import ml_dtypes

    flag = np.asarray(flag)
    cs = tuple(float(np.asarray(log_scale)[0]) * float(a) for a in np.asarray(alpha))

    f8 = ml_dtypes.float8_e4m3
    dist8 = np.ascontiguousarray(np.asarray(dist).transpose(0, 2, 1)).astype(f8)
    dist8 = dist8.reshape(B, TC, TCS, S)
    node_t = np.ascontiguousarray(np.asarray(node_xy_demand).transpose(0, 2, 1)).astype(np.float16)
    depot = np.asarray(depot_xy).reshape(B, 2).astype(np.float32)
    flagf = flag.astype(np.float32)

    f16 = lambda a: np.ascontiguousarray(np.asarray(a)).astype(np.float16)
    f32 = lambda a: np.ascontiguousarray(np.asarray(a)).astype(np.float32)
    w2t = np.asarray(W2).transpose(0, 2, 1).reshape(L, FC, P, D).transpose(0, 2, 1, 3)
    shared = {
        "wqt": f16(np.asarray(Wq).transpose(0, 2, 1)),
        "wkt": f16(np.asarray(Wk).transpose(0, 2, 1)),
        "wvt": f16(np.asarray(Wv).transpose(0, 2, 1)),
        "w1t": f16(np.asarray(W1).transpose(0, 2, 1)),
        "w2t": np.ascontiguousarray(w2t).astype(np.float16),
        "wnt": f16(np.asarray(Wn).T),
        "wdt": f32(np.asarray(Wd).T),
        "wint": f32(np.asarray(Win).T),
        "woutt": f32(np.asarray(Wout).T),
        "biases4": f32(np.stack([np.asarray(bd), np.asarray(bn),
                                 np.asarray(bin_), np.asarray(bout)], axis=1)),
        "bw1_t": f32(np.asarray(bW1).reshape(L, FC, P).transpose(2, 0, 1)),
        "g1_t": f32(np.asarray(g1).T),
        "b1_t": f32(np.asarray(b1).T),
        "g2_t": f32(np.asarray(g2).T),
        "b2_t": f32(np.asarray(b2).T),
    }
    in_maps = []
    for c in range(NCORES):
        sl = slice(c * IPC, (c + 1) * IPC)
        m = dict(shared)
        m["dist8"] = dist8[sl]
        m["node_t"] = node_t[sl]
        m["depot"] = depot[sl]
        m["flagf"] = flagf[sl]
        in_maps.append(m)
    return cs, in_maps


TRACE = False
LAST_RESULT = None


def kernel(**inputs):
    global LAST_RESULT
    cs, in_maps = prep_inputs(**inputs)
    nc = _get_nc(cs)
    res = run_bass_kernel_spmd(nc, in_maps, list(range(NCORES)), trace=TRACE)
    LAST_RESULT = res
    out = np.concatenate([r["out32"] for r in res.results], axis=0)  # [B, D, S]
    return np.ascontiguousarray(out.transpose(0, 2, 1)).astype(np.float32)


# revision 46
# speedup vs baseline: 1.0318x; 1.0318x over previous
"""Trainium2 Bass kernel for nn_CVRP_Encoder (AFT-style CVRP encoder).

Data-parallel over batch B=32 across 8 NeuronCores (4 items/core). Per item
everything lives in a transposed [D=128 (partitions), S=1000 (free)] layout so
instance-norm reduces along the free axis. S splits into 8 chunks of 125 for
the attention contraction (t on partitions) and 2 chunks of 500 for matmul
free dims.

Perf structure:
 - the big attention contractions (es@ek, es@ekv) run as fp8 DoubleRow
   matmuls (2 contraction rows per PE cycle): es in e4m3, ek/ekv in e5m2
   with a constant K-shift (ek' = exp(k - K0); e^-K0 cancels in num/den).
   The fp8 noise also largely cancels in num/den, measured ~0 extra error.
 - FFN stays f16 end-to-end (fp8 noise there hits the residual directly).
 - the residual stream is f16; norm chains use native scalar_tensor_tensor
   ops with fused sum accumulators: aft2 = (tanh(q/2)+1)*wgt (the sigmoid
   affine folded in), y = aft2*0.5 + x (accum sum(y)), y2' = A1*y + ff
   (accum sum(y2')).  C1 and bW2 are dropped: per-channel shifts cancel in
   the next instance norm (shift invariance).
 - sigmoid via tanh keeps every activation (tanh/exp/square/relu/identity)
   in the single `exp_and_others` hw table set: no table reloads.
 - relu passes split between scalar and vector engines; the normalized-x
   applications run on gpsimd (SBUF-only engine).
 - items are processed in norm-groups of 2, each group's norm chain emitted
   before the next group's elementwise work so it overlaps matmuls.
"""
import sys

sys.path.insert(0, "/opt/trn_rl_repo")

import numpy as np

import concourse.bass as bass
import concourse.tile as tile
from concourse import bacc, mybir
from concourse.bass_utils import run_bass_kernel_spmd

F32 = mybir.dt.float32
F16 = mybir.dt.float16
BF16 = mybir.dt.bfloat16
F8 = mybir.dt.float8e4
F8E5 = mybir.dt.float8e5
I32 = mybir.dt.int32
AF = mybir.ActivationFunctionType
ALU = mybir.AluOpType
DR = mybir.MatmulPerfMode.DoubleRow

B, N, D, F, L = 32, 999, 128, 512, 6
S = N + 1
P = 128
NCORES = 8
IPC = B // NCORES
TC = 8
TCS = S // TC      # 125
SC = 2
SCS = S // SC      # 500
FC = F // P        # 4
EPS = 1e-5
RSQRT_MAGIC = 0x5F3759DF + 1
GRP = 2            # items per norm-batching group
K0 = 3.5           # ek exponent shift (cancels in num/den)


def _bcast_dram(handle, n_part, idx, count):
    ap = handle[:]
    return bass.AP(tensor=ap.tensor, offset=idx, ap=[[0, n_part], [1, count]])


def _nv(t):
    """[P, 1024] tile/psum -> [P, 2, 500] strided view (skip 512-pad)."""
    return t[:].rearrange("p (n s) -> p n s", n=2)[:, :, 0:SCS]


def _v2(t):
    """[P, S] tile -> [P, 2, 500] view."""
    return t[:].rearrange("p (n s) -> p n s", n=2)


def _ckv(t):
    """[P, 1024] psum -> [P, 8, 128] chunk view."""
    return t[:].rearrange("p (c d) -> p c d", c=TC)


def build_cvrp(cs):
    """cs: per-layer scale constants c_l = log_scale * alpha[l]."""
    shared_es = all(abs(c - cs[0]) < 1e-30 for c in cs)

    nc = bacc.Bacc("TRN2", target_bir_lowering=False, debug=False,
                   num_devices=NCORES)

    g = {}
    g["dist8"] = nc.declare_dram_parameter("dist8", [IPC, TC, TCS, S], F8, isOutput=False)
    g["node_t"] = nc.declare_dram_parameter("node_t", [IPC, 3, N], F16, isOutput=False)
    g["depot"] = nc.declare_dram_parameter("depot", [IPC, 2], F32, isOutput=False)
    g["flagf"] = nc.declare_dram_parameter("flagf", [IPC], F32, isOutput=False)
    g["wqt"] = nc.declare_dram_parameter("wqt", [L, D, D], F16, isOutput=False)
    g["wkt"] = nc.declare_dram_parameter("wkt", [L, D, D], F16, isOutput=False)
    g["wvt"] = nc.declare_dram_parameter("wvt", [L, D, D], F16, isOutput=False)
    g["w1t"] = nc.declare_dram_parameter("w1t", [L, D, F], F16, isOutput=False)
    g["w2t"] = nc.declare_dram_parameter("w2t", [L, P, FC, D], F16, isOutput=False)
    g["wnt"] = nc.declare_dram_parameter("wnt", [3, D], F16, isOutput=False)
    g["wdt"] = nc.declare_dram_parameter("wdt", [2, D], F32, isOutput=False)
    g["wint"] = nc.declare_dram_parameter("wint", [D, D], F32, isOutput=False)
    g["woutt"] = nc.declare_dram_parameter("woutt", [D, D], F32, isOutput=False)
    g["biases4"] = nc.declare_dram_parameter("biases4", [D, 4], F32, isOutput=False)
    g["bw1_t"] = nc.declare_dram_parameter("bw1_t", [D, L, FC], F32, isOutput=False)
    g["g1_t"] = nc.declare_dram_parameter("g1_t", [D, L], F32, isOutput=False)
    g["b1_t"] = nc.declare_dram_parameter("b1_t", [D, L], F32, isOutput=False)
    g["g2_t"] = nc.declare_dram_parameter("g2_t", [D, L], F32, isOutput=False)
    g["b2_t"] = nc.declare_dram_parameter("b2_t", [D, L], F32, isOutput=False)
    g["out32"] = nc.declare_dram_parameter("out32", [IPC, D, S], F32, isOutput=True)

    with tile.TileContext(nc) as tc_ctx:
        _body(nc, tc_ctx, g, cs, shared_es)
    nc.compile()
    return nc


def _norm_smalls(nc, np_, sums, sumsq, g_col, b_col, tag, sum_scale, sq_scale,
                 sx=None, extras=None):
    """Instance-norm scalar math on [D, GRP] tiles.
    mean = (sums [+ sx]) * sum_scale; var = sumsq * sq_scale + eps - mean^2;
    rstd via bit-trick + 2 Newton iters. Returns (A, C): out = A*y + C for the
    TRUE-scale y. extras='n1' also returns Ah = A*0.5 (applies A to the 2x
    tensor); extras='n2' also returns (A2d, C2d) = (2A, 2C) and writes
    sx_next = A2d*sums + S*C2d into the provided column."""
    sm = np_.tile([D, 12, GRP], F32, tag=f"nsm_{tag}")
    mean, msq, var = sm[:, 0], sm[:, 1], sm[:, 2]
    if sx is not None:
        nc.vector.tensor_tensor(mean, sums, sx, ALU.add)
        nc.vector.tensor_scalar(mean, mean, sum_scale, None, ALU.mult)
    else:
        nc.vector.tensor_scalar(mean, sums, sum_scale, None, ALU.mult)
    nc.vector.tensor_tensor(msq, mean, mean, ALU.mult)
    nc.vector.tensor_scalar(var, sumsq, sq_scale, EPS, ALU.mult, ALU.add)
    nc.vector.tensor_tensor(var, var, msq, ALU.subtract)
    ry = sm[:, 3]
    ibits = ry.bitcast(I32)
    nc.vector.tensor_scalar(ibits, var.bitcast(I32), 1, -1,
                            ALU.logical_shift_right, ALU.bitwise_xor)
    nc.vector.tensor_scalar(ibits, ibits, RSQRT_MAGIC, None, ALU.add)
    t1, t2 = sm[:, 4], sm[:, 5]
    for _ in range(2):
        nc.vector.tensor_tensor(t1, ry, ry, ALU.mult)
        nc.vector.scalar_tensor_tensor(out=t2, in0=t1, scalar=-0.5, in1=var,
                                       op0=ALU.mult, op1=ALU.mult)
        nc.vector.scalar_tensor_tensor(out=ry, in0=t2, scalar=1.5, in1=ry,
                                       op0=ALU.add, op1=ALU.mult)
    A, C = sm[:, 6], sm[:, 7]
    nc.vector.tensor_scalar(A, ry, g_col, None, ALU.mult)
    nc.vector.tensor_tensor(C, mean, A, ALU.mult)
    nc.vector.tensor_scalar(C, C, b_col, -1.0, ALU.subtract, ALU.mult)
    if extras == "n1":
        Ah = sm[:, 8]
        nc.vector.tensor_scalar(Ah, A, 0.5, None, ALU.mult)
        return A, C, Ah
    if extras == "n2":
        A2d, C2d, sx_next, tmp = sm[:, 8], sm[:, 9], sm[:, 10], sm[:, 11]
        nc.vector.tensor_scalar(A2d, A, 2.0, None, ALU.mult)
        nc.vector.tensor_scalar(C2d, C, 2.0, None, ALU.mult)
        nc.vector.tensor_tensor(sx_next, A2d, sums, ALU.mult)
        nc.vector.tensor_scalar(tmp, C2d, float(S), None, ALU.mult)
        nc.vector.tensor_tensor(sx_next, sx_next, tmp, ALU.add)
        return A, C, A2d, C2d, sx_next
    return A, C


DBG = None      # e.g. ("y", 0): dump tile for each item at layer 0, then stop
DBG_STOP = [False]


def _dbg_dump(nc, g, xpool, name, l, i, ap):
    """Dump a 2D ap [p, f] (f<=1000) into out32[i][:p, :f] for debugging."""
    if DBG is None or DBG != (name, l):
        return
    p, f = ap.shape[0], ap.shape[-1]
    t = xpool.tile([D, S], F32, tag=f"dbg_{i}")
    nc.vector.tensor_scalar(t[:p, 0:f], ap, 1.0, None, ALU.mult)
    nc.sync.dma_start(g["out32"][i][:p, 0:f], t[:p, 0:f])
    if i == IPC - 1:
        DBG_STOP[0] = True


def _body(nc, tc, g, cs, shared_es):
    from contextlib import ExitStack

    ctx = ExitStack()
    singles = ctx.enter_context(tc.tile_pool(name="singles", bufs=1))
    xpool = ctx.enter_context(tc.tile_pool(name="xpool", bufs=1))
    tp = ctx.enter_context(tc.tile_pool(name="tp", bufs=2))
    scr = ctx.enter_context(tc.tile_pool(name="scr", bufs=2))
    np_ = ctx.enter_context(tc.tile_pool(name="npool", bufs=2))
    pp = ctx.enter_context(tc.tile_pool(name="pp", bufs=1))
    ps = ctx.enter_context(tc.tile_pool(name="ps", bufs=4, space="PSUM"))

    # ---- resident weights: L0-critical tensors on the SP queue (shortest
    # path), everything else on the Activation DMA queue so the startup
    # descriptor burst doesn't delay the embedding.
    t_wnt = singles.tile([3, D], F16, tag="wnt")
    nc.sync.dma_start(t_wnt[:], g["wnt"][:])
    t_wdt = singles.tile([2, D], F32, tag="wdt")
    nc.sync.dma_start(t_wdt[:], g["wdt"][:])
    t_wint = singles.tile([D, D], F32, tag="wint")
    nc.sync.dma_start(t_wint[:], g["wint"][:])
    t_woutt = singles.tile([D, D], F32, tag="woutt")
    nc.sync.dma_start(t_woutt[:], g["woutt"][:])
    sm_t = {}
    for nm, shp in (("biases4", [D, 4]), ("bw1_t", [D, L, FC]), ("g1_t", [D, L]),
                    ("b1_t", [D, L]), ("g2_t", [D, L]), ("b2_t", [D, L])):
        t = singles.tile(shp, F32, tag=nm)
        nc.sync.dma_start(t[:], g[nm][:])
        sm_t[nm] = t
    t_b4, t_bw1 = sm_t["biases4"], sm_t["bw1_t"]
    t_g1, t_b1, t_g2, t_b2 = sm_t["g1_t"], sm_t["b1_t"], sm_t["g2_t"], sm_t["b2_t"]
    t_ff = singles.tile([P, IPC], F32, tag="ffl")
    nc.sync.dma_start(t_ff[:], _bcast_dram(g["flagf"], P, 0, IPC))
    t_ksh = singles.tile([P, 1], F32, tag="kshift")
    nc.any.memset(t_ksh[:], -K0)
    t_wqt, t_wkt, t_wvt, t_w1t, t_w2t = [], [], [], [], []
    wspecs = ((t_wqt, g["wqt"], [D, D]), (t_wkt, g["wkt"], [D, D]),
              (t_wvt, g["wvt"], [D, D]), (t_w1t, g["w1t"], [D, F]),
              (t_w2t, g["w2t"], [P, FC, D]))
    for l in range(L):
        for lst, src, shape in wspecs:
            w = singles.tile(shape, F16, tag=f"w{id(lst)}_{l}")
            if l == 0:
                nc.sync.dma_start(w[:], src[l])
            lst.append(w)

    BD, BN_, BIN, BOUT = (t_b4[:, i : i + 1] for i in range(4))

    # ---- embedding ----
    x32s = []
    for i in range(IPC):
        x32 = xpool.tile([D, S], F32, tag=f"x32_{i}")
        t_node = scr.tile([P, 1024], F16, tag="node16")
        nc.sync.dma_start(t_node[:3, 0:N], g["node_t"][i])
        t_dep = tp.tile([2, 1], F32, tag="dep")
        nc.sync.dma_start(t_dep[:], g["depot"][i, :, None])
        pe = ps.tile([P, 1024], F32, tag="ps")
        nc.tensor.matmul(pe[:, 0:500], t_wnt[:], t_node[:3, 0:500], start=True, stop=True)
        nc.tensor.matmul(pe[:, 512:1011], t_wnt[:], t_node[:3, 500:999], start=True, stop=True)
        nc.scalar.activation(x32[:, 1:501], pe[:, 0:500], AF.Identity, bias=BN_, scale=1.0)
        nc.scalar.activation(x32[:, 501:1000], pe[:, 512:1011], AF.Identity, bias=BN_, scale=1.0)
        pd = ps.tile([P, 1024], F32, tag="ps")
        nc.tensor.matmul(pd[:, 0:1], t_wdt[:], t_dep[:], start=True, stop=True)
        nc.scalar.activation(x32[:, 0:1], pd[:, 0:1], AF.Identity, bias=BD, scale=1.0)
        pw = ps.tile([P, 1024], F32, tag="ps")
        nc.tensor.matmul(pw[:, 0:1], t_wint[:], x32[:, 1:2], start=True, stop=True)
        nc.scalar.activation(x32[:, 1:2], pw[:, 0:1], AF.Identity, bias=BIN, scale=1.0)
        # flag row fix: u = f*x0 + (1-f)*x999 ; w = Wout@u + bout ;
        # x0 += f*(w-u) ; x999 += (1-f)*(w-u)
        fcol = t_ff[:, i : i + 1]
        sm = np_.tile([D, 8], F32, tag="flagtmp")
        d1, u, t2, w_sb, d0 = (sm[:, j : j + 1] for j in range(5))
        nc.vector.tensor_tensor(d1, x32[:, 0:1], x32[:, 999:1000], ALU.subtract)
        nc.vector.tensor_scalar(d1, d1, fcol, None, ALU.mult)
        nc.vector.tensor_tensor(u, x32[:, 999:1000], d1, ALU.add)
        pf = ps.tile([P, 1024], F32, tag="ps")
        nc.tensor.matmul(pf[:, 0:1], t_woutt[:], u, start=True, stop=True)
        nc.scalar.activation(w_sb, pf[:, 0:1], AF.Identity, bias=BOUT, scale=1.0)
        nc.vector.tensor_tensor(t2, w_sb, u, ALU.subtract)
        nc.vector.tensor_scalar(d0, t2, fcol, None, ALU.mult)
        nc.vector.tensor_tensor(x32[:, 0:1], x32[:, 0:1], d0, ALU.add)
        nc.vector.tensor_tensor(x32[:, 999:1000], x32[:, 999:1000], t2, ALU.add)
        nc.vector.tensor_tensor(x32[:, 999:1000], x32[:, 999:1000], d0, ALU.subtract)
        x32s.append(x32)

    # The normalized stream is carried DOUBLED (x2x = 2x): the factor folds
    # into activation scales and norm constants for free, which lets the
    # y-residual add run as a plain gpsimd tensor_tensor.
    # sx_all[:, i] = sum_s(x2x) per item, carried for the norm-1 mean.
    sx_all = singles.tile([D, IPC], F32, tag="sxall")
    x16s = []
    for i in range(IPC):
        x16 = xpool.tile([D, S], F16, tag=f"x16_{i}")
        nc.gpsimd.tensor_scalar(x16[:], x32s[i][:], 2.0, None, ALU.mult)
        nc.vector.tensor_reduce(sx_all[:, i : i + 1], x32s[i][:],
                                axis=mybir.AxisListType.X, op=ALU.add)
        x16s.append(x16)
    nc.vector.tensor_scalar(sx_all[:], sx_all[:], 2.0, None, ALU.mult)

    for l in range(1, L):
        for li, (lst, src, shape) in enumerate(wspecs):
            nc.gpsimd.dma_start(lst[l][:], src[l])

    es_tiles = {}
    pending_norm2 = [None]

    # ---- encoder layers ----
    for l in range(L):
        sigs_tq, wgts, ys, h16s, pf2s, y2s, eks = {}, {}, {}, {}, {}, {}, {}
        sts = {}

        def attn_qkv(i):
            x16 = x16s[i]       # 2x-scaled stream; folded into scales below
            pq = ps.tile([P, 1024], F32, tag="ps")
            nc.tensor.matmul(pq[:, 0:500], t_wqt[l][:], x16[:, 0:500], start=True, stop=True)
            nc.tensor.matmul(pq[:, 512:1012], t_wqt[l][:], x16[:, 500:1000], start=True, stop=True)
            pk = ps.tile([P, 1024], F32, tag="ps")
            pv = ps.tile([P, 1024], F32, tag="ps")
            for c in range(TC):
                lhs = x16[:, c * TCS : (c + 1) * TCS]
                nc.tensor.matmul(pk[:TCS, c * P : (c + 1) * P], lhs, t_wkt[l][:], start=True, stop=True)
                nc.tensor.matmul(pv[:TCS, c * P : (c + 1) * P], lhs, t_wvt[l][:], start=True, stop=True)
            # K-shift: ek' = exp(k - K0); e^-K0 scales num and den identically.
            ek8 = tp.tile([P, TC, P], F8E5, tag=f"ek8_{i % 2}")
            nc.scalar.activation(ek8[:TCS], _ckv(pk)[:TCS], AF.Exp,
                                 bias=t_ksh[:TCS], scale=0.5)
            tq = pp.tile([P, S], F16, tag=f"tq_{i}")
            nc.scalar.activation(_v2(tq), _nv(pq), AF.Tanh, bias=0.0, scale=0.25)
            sigs_tq[i] = tq
            ekv8 = tp.tile([P, TC, P], F8E5, tag=f"ekv8_{i % 2}")
            nc.vector.scalar_tensor_tensor(
                out=ekv8[:TCS], in0=_ckv(pv)[:TCS], scalar=0.5, in1=ek8[:TCS],
                op0=ALU.mult, op1=ALU.mult)
            eks[i] = (ek8, ekv8)

        def attn_dennum(i):
            ek8, ekv8 = eks[i]
            if shared_es:
                if i not in es_tiles:
                    es = singles.tile([P, TC, S], F8, tag=f"es{i}", name=f"es{i}")
                    for h in range(SC):
                        hs = slice(h * SCS, (h + 1) * SCS)
                        nc.sync.dma_start(
                            es[:TCS, :, hs],
                            g["dist8"][i, :, :, hs].rearrange("c p s -> p c s"))
                        nc.scalar.activation(es[:TCS, :, hs], es[:TCS, :, hs],
                                             AF.Exp, bias=0.0, scale=-cs[0])
                    es_tiles[i] = es
                es = es_tiles[i]
            else:
                if i not in es_tiles:
                    raw = singles.tile([P, TC, S], F8, tag=f"esr{i}", name=f"esr{i}")
                    nc.sync.dma_start(raw[:TCS, 0:4], g["dist8"][i, 0:4].rearrange("c p s -> p c s"))
                    nc.sync.dma_start(raw[:TCS, 4:8], g["dist8"][i, 4:8].rearrange("c p s -> p c s"))
                    es_tiles[i] = raw
                es = tp.tile([P, TC, S], F8, tag="es_dyn")
                nc.scalar.activation(es[:TCS], es_tiles[i][:TCS], AF.Exp,
                                     bias=0.0, scale=-cs[l])
            pden = ps.tile([P, 1024], F32, tag="ps")
            for sc in range(SC):
                off = sc * 512
                ssl = slice(sc * SCS, (sc + 1) * SCS)
                for cp in range(TC // 2):
                    nc.tensor.matmul(pden[:, off : off + SCS],
                                     ek8[:TCS, 2 * cp : 2 * cp + 2, :],
                                     es[:TCS, 2 * cp : 2 * cp + 2, ssl],
                                     start=(cp == 0), stop=(cp == TC // 2 - 1),
                                     perf_mode=DR)
            pnum = ps.tile([P, 1024], F32, tag="ps")
            for sc in range(SC):
                off = sc * 512
                ssl = slice(sc * SCS, (sc + 1) * SCS)
                for cp in range(TC // 2):
                    nc.tensor.matmul(pnum[:, off : off + SCS],
                                     ekv8[:TCS, 2 * cp : 2 * cp + 2, :],
                                     es[:TCS, 2 * cp : 2 * cp + 2, ssl],
                                     start=(cp == 0), stop=(cp == TC // 2 - 1),
                                     perf_mode=DR)
            rden = scr.tile([P, 1024], F32, tag="rden")
            nc.vector.reciprocal_approx_fast(out=_nv(rden), in_=_nv(pden))
            wgt = pp.tile([P, S], F16, tag=f"wgt_{i}")
            nc.vector.tensor_tensor(_v2(wgt), _nv(pnum), _nv(rden), ALU.mult)
            wgts[i] = wgt
            _dbg_dump(nc, g, xpool, "x16", l, i, x16s[i][:])
            _dbg_dump(nc, g, xpool, "es", l, i, es[:TCS, 0, :])
            _dbg_dump(nc, g, xpool, "tq", l, i, sigs_tq[i][:])
            _dbg_dump(nc, g, xpool, "rden", l, i, rden[:, 0:500])
            _dbg_dump(nc, g, xpool, "wgt", l, i, wgt[:])

        def leafs(i):
            g0 = (i // GRP) * GRP
            if g0 not in sts:
                sts[g0] = np_.tile([D, 4, GRP], F32, tag=f"st_{g0}", name=f"st_{g0}")
            st1 = sts[g0]
            j = i - g0
            # aft2 = (tanh(q/2)+1)*wgt = 2*sigmoid(q)*weighted
            aft2 = tp.tile([P, S], F16, tag="aft")
            nc.vector.scalar_tensor_tensor(
                out=aft2[:], in0=sigs_tq[i][:], scalar=1.0, in1=wgts[i][:],
                op0=ALU.add, op1=ALU.mult, accum_out=st1[:, 0, j : j + 1])
            # y2x = 2y = aft2 + x2x  (sum comes analytically: Saft2 + sx)
            y = pp.tile([P, S], F16, tag=f"y_{i}")
            nc.gpsimd.tensor_tensor(y[:], aft2[:], x16s[i][:], ALU.add)
            ys[i] = y
            sq = scr.tile([P, 1024], F32, tag="scr4k")
            nc.scalar.activation(sq[:, 0:S], y[:], AF.Square, accum_out=st1[:, 1, j : j + 1])

        def norm1(g0):
            st1 = sts[g0]
            A1, C1, A1h = _norm_smalls(
                nc, np_, st1[:, 0], st1[:, 1],
                t_g1[:, l : l + 1], t_b1[:, l : l + 1], f"n1_{g0}",
                sum_scale=1.0 / (2 * S), sq_scale=1.0 / (4 * S),
                sx=sx_all[:, g0 : g0 + GRP], extras="n1")
            sts[f"AC{g0}"] = (A1, C1, A1h)
            for i in range(g0, g0 + GRP):
                j = i - g0
                h16 = pp.tile([P, S], F16, tag=f"h16_{i}")
                nc.gpsimd.tensor_scalar(h16[:], ys[i][:], A1h[:, j : j + 1], C1[:, j : j + 1],
                                        ALU.mult, ALU.add)
                h16s[i] = h16
                _dbg_dump(nc, g, xpool, "y", l, i, ys[i][:])
                _dbg_dump(nc, g, xpool, "h", l, i, h16[:])

        def ffn(i):
            h16 = h16s[i]
            pf2 = ps.tile([P, 1024], F32, tag="ps")
            r16 = tp.tile([P, FC, S], F16, tag="r16")
            r16v = r16[:].rearrange("p f (n s) -> p f n s", n=2)
            for fc in range(FC):
                pf1 = ps.tile([P, 1024], F32, tag="ps")
                w1 = t_w1t[l][:, fc * P : (fc + 1) * P]
                nc.tensor.matmul(pf1[:, 0:500], w1, h16[:, 0:500], start=True, stop=True)
                nc.tensor.matmul(pf1[:, 512:1012], w1, h16[:, 500:1000], start=True, stop=True)
                bcol = t_bw1[:, l, fc : fc + 1]
                if fc < 3:
                    nc.scalar.activation(r16v[:, fc], _nv(pf1), AF.Relu, bias=bcol, scale=1.0)
                else:
                    nc.vector.tensor_scalar(r16v[:, fc], _nv(pf1), bcol, 0.0, ALU.add, ALU.max)
            for sc in range(SC):
                off = sc * 512
                ssl = slice(sc * SCS, (sc + 1) * SCS)
                for fc in range(FC):
                    nc.tensor.matmul(pf2[:, off : off + SCS],
                                     t_w2t[l][:, fc, :], r16[:, fc, ssl],
                                     start=(fc == 0), stop=(fc == FC - 1))
            pf2s[i] = pf2

        def y2p(i):
            g0 = (i // GRP) * GRP
            st1 = sts[g0]
            A1, C1, A1h = sts[f"AC{g0}"]
            j = i - g0
            # y2' = A1*y + ff  (C1 and bW2 shifts cancel in this norm)
            y2 = pp.tile([P, S], F16, tag=f"y2_{i}")
            nc.vector.scalar_tensor_tensor(
                out=_v2(y2), in0=_v2(ys[i]), scalar=A1h[:, j : j + 1],
                in1=_nv(pf2s[i]), op0=ALU.mult, op1=ALU.add,
                accum_out=st1[:, 2, j : j + 1])
            y2s[i] = y2
            _dbg_dump(nc, g, xpool, "y2", l, i, y2[:])
            sq = scr.tile([P, 1024], F32, tag="scr4k")
            nc.scalar.activation(sq[:, 0:S], y2[:], AF.Square, accum_out=st1[:, 3, j : j + 1])

        def norm2(g0, sts=sts, y2s=y2s, l=l):
            st1 = sts[g0]
            A2, C2, A2d, C2d, sxn = _norm_smalls(
                nc, np_, st1[:, 2], st1[:, 3],
                t_g2[:, l : l + 1], t_b2[:, l : l + 1], f"n2_{g0}",
                sum_scale=1.0 / S, sq_scale=1.0 / S, extras="n2")
            nc.vector.tensor_scalar(sx_all[:, g0 : g0 + GRP], sxn, 1.0, None, ALU.mult)
            for i in range(g0, g0 + GRP):
                j = i - g0
                if l < L - 1:
                    nx16 = xpool.tile([D, S], F16, tag=f"x16_{i}{'b' if l % 2 == 0 else ''}")
                    nc.gpsimd.tensor_scalar(nx16[:], y2s[i][:], A2d[:, j : j + 1], C2d[:, j : j + 1],
                                            ALU.mult, ALU.add)
                    x16s[i] = nx16
                else:
                    xout = xpool.tile([D, S], F32, tag=f"x32_{i}")
                    nc.gpsimd.tensor_scalar(xout[:], y2s[i][:], A2[:, j : j + 1], C2[:, j : j + 1],
                                            ALU.mult, ALU.add)
                    nc.sync.dma_start(g["out32"][i], xout[:])

        if DBG_STOP[0]:
            break
        # Software-pipelined emission: all qkv matmuls run ahead of den/num so
        # the PE never starves; every cross-engine wait has independent work
        # queued in front of it on the same engine queue.
        lo_groups = [g0 for g0 in range(0, IPC // 2, GRP)]
        hi_groups = [g0 for g0 in range(IPC // 2, IPC, GRP)]
        attn_qkv(0); attn_dennum(0)
        attn_qkv(1); attn_dennum(1)
        if pending_norm2[0] is not None:
            pending_norm2[0]()          # prev layer norm2(hi) + x2x(2,3)
            pending_norm2[0] = None
        leafs(0); leafs(1)
        attn_qkv(2); attn_dennum(2)
        attn_qkv(3); attn_dennum(3)
        for g0 in lo_groups:
            norm1(g0)
        leafs(2); leafs(3)
        ffn(0); ffn(1)
        for g0 in hi_groups:
            norm1(g0)
        y2p(0); y2p(1)
        for g0 in lo_groups:
            norm2(g0)
        ffn(2); ffn(3)
        y2p(2); y2p(3)
        n2 = norm2
        pending_norm2[0] = (lambda n2=n2, gs=tuple(hi_groups): [n2(g0) for g0 in gs])

    if pending_norm2[0] is not None and not DBG_STOP[0]:
        pending_norm2[0]()
    ctx.close()


# ------------------------------------------------------------------
# host wrapper
# ------------------------------------------------------------------
_cache = {}


def _get_nc(cs_key):
    if cs_key not in _cache:
        _cache[cs_key] = build_cvrp(list(cs_key))
    return _cache[cs_key]


def prep_inputs(depot_xy, node_xy_demand, dist, log_scale, flag,
                Wd, bd, Wn, bn, Win, bin_, Wout, bout,
                Wq, Wk, Wv, alpha, g1, b1, W1, bW1, W2, bW2, g2, b2):
    import ml_dtypes

    flag = np.asarray(flag)
    cs = tuple(float(np.asarray(log_scale)[0]) * float(a) for a in np.asarray(alpha))

    f8 = ml_dtypes.float8_e4m3
    dist8 = np.ascontiguousarray(np.asarray(dist).transpose(0, 2, 1)).astype(f8)
    dist8 = dist8.reshape(B, TC, TCS, S)
    node_t = np.ascontiguousarray(np.asarray(node_xy_demand).transpose(0, 2, 1)).astype(np.float16)
    depot = np.asarray(depot_xy).reshape(B, 2).astype(np.float32)
    flagf = flag.astype(np.float32)

    f16 = lambda a: np.ascontiguousarray(np.asarray(a)).astype(np.float16)
    f32 = lambda a: np.ascontiguousarray(np.asarray(a)).astype(np.float32)
    w2t = np.asarray(W2).transpose(0, 2, 1).reshape(L, FC, P, D).transpose(0, 2, 1, 3)
    shared = {
        "wqt": f16(np.asarray(Wq).transpose(0, 2, 1)),
        "wkt": f16(np.asarray(Wk).transpose(0, 2, 1)),
        "wvt": f16(np.asarray(Wv).transpose(0, 2, 1)),
        "w1t": f16(np.asarray(W1).transpose(0, 2, 1)),
        "w2t": np.ascontiguousarray(w2t).astype(np.float16),
        "wnt": f16(np.asarray(Wn).T),
        "wdt": f32(np.asarray(Wd).T),
        "wint": f32(np.asarray(Win).T),
        "woutt": f32(np.asarray(Wout).T),
        "biases4": f32(np.stack([np.asarray(bd), np.asarray(bn),
                                 np.asarray(bin_), np.asarray(bout)], axis=1)),
        "bw1_t": f32(np.asarray(bW1).reshape(L, FC, P).transpose(2, 0, 1)),
        "g1_t": f32(np.asarray(g1).T),
        "b1_t": f32(np.asarray(b1).T),
        "g2_t": f32(np.asarray(g2).T),
        "b2_t": f32(np.asarray(b2).T),
    }
    in_maps = []
    for c in range(NCORES):
        sl = slice(c * IPC, (c + 1) * IPC)
        m = dict(shared)
        m["dist8"] = dist8[sl]
        m["node_t"] = node_t[sl]
        m["depot"] = depot[sl]
        m["flagf"] = flagf[sl]
        in_maps.append(m)
    return cs, in_maps


TRACE = False
LAST_RESULT = None


def kernel(**inputs):
    global LAST_RESULT
    cs, in_maps = prep_inputs(**inputs)
    nc = _get_nc(cs)
    res = run_bass_kernel_spmd(nc, in_maps, list(range(NCORES)), trace=TRACE)
    LAST_RESULT = res
    out = np.concatenate([r["out32"] for r in res.results], axis=0)  # [B, D, S]
    return np.ascontiguousarray(out.transpose(0, 2, 1)).astype(np.float32)
